# revision 2
# baseline (speedup 1.0000x reference)
"""Trainium2 Bass kernel for adjacency-masked multi-head attention.

Problem (fixed shapes): x[4,2048,128], A[2048,2048] int32 0/1, Wq[128,128],
Wkv[256,128], Wp[128,128], bp[128]; out = softmax-attention with mask + resid.

Sharding: 8 cores = (batch b in 0..3) x (query half s in 0..1). Each core
computes K/V for its whole batch and attention for its 1024 queries.

Kernel design (per core):
- Scores prescaled by ALPHA = 2^7*log2(e), folded into Wq on the host.
- The adjacency mask ships once as fp8e5m2 (A-1)*16384, duplicated per
  head-half so engines read 1024 contiguous columns.
- Two eviction classes for the 64 [128,1024] score tiles:
    C: PE accumulates the mask bias into score PSUM via fp8 identity
       matmuls; a single ACT exp (free affine: scale 1/ALPHA, bias ln F)
       then emits masked bf16 p directly - no separate mask multiply.
    D: a single DVE scalar_tensor_tensor computes
       i16 = s*ALPHA + MAGIC + (A-1)*16384, written as int16 and bitcast
       bf16 == exp(s)*F (Schraudolph fast-exp, ~6% max rel err; the constant
       F cancels in softmax; masked entries land in the denormal range).
- PV runs with 64-wide aug-[v | ones] stationary tiles so numerator and
  denominator emerge from one accumulation stream; lag-2 software pipelining
  keeps PE fed while evictions retire; the normalization/projection stage is
  deferred into the next query-chunk's loop to avoid a PSUM stall.
"""

import math

import numpy as np

_CACHE = {}

B, N, C, H, HD = 4, 2048, 128, 4, 32
NQ = 1024
KB = N // 128
QC = NQ // 512
SCALE = HD ** -0.5

ALPHA = 128.0 / math.log(2.0)
MAGIC = 16514.5
AMASK = 16384.0
F16 = 2.0 ** ((MAGIC - 16256.0) / 128.0)
B0 = math.log(F16)


def _tile_class(qc, kb, hp):
    if hp == 0:
        return "C"
    return "C" if kb == 7 else "D"


def _build():
    import concourse.bacc as bacc
    import concourse.mybir as mybir
    import concourse.tile as tile
    from concourse.tile_rust import add_dep_helper

    F32 = mybir.dt.float32
    BF16 = mybir.dt.bfloat16
    FP8E5 = mybir.dt.float8e5
    I16 = mybir.dt.int16
    EXP = mybir.ActivationFunctionType.Exp
    CPY = mybir.ActivationFunctionType.Copy
    ADD = mybir.AluOpType.add
    MULT = mybir.AluOpType.mult

    nc = bacc.Bacc("TRN2", target_bir_lowering=False, debug=False)

    xT = nc.dram_tensor("xT", [C, N], BF16, kind="ExternalInput")
    xqT = nc.dram_tensor("xqT", [C, NQ], F32, kind="ExternalInput")
    AT16 = nc.dram_tensor("AT16", [N, 2 * NQ], FP8E5, kind="ExternalInput")
    WqT = nc.dram_tensor("WqT", [C, C], BF16, kind="ExternalInput")
    WkT = nc.dram_tensor("WkT", [C, C], BF16, kind="ExternalInput")
    WvT = nc.dram_tensor("WvT", [C, C], BF16, kind="ExternalInput")
    IDENT = nc.dram_tensor("IDENT", [C, C], FP8E5, kind="ExternalInput")
    SEL = nc.dram_tensor("SEL", [C, C], BF16, kind="ExternalInput")
    WpT0 = nc.dram_tensor("WpT0", [C, C], BF16, kind="ExternalInput")
    WpT1 = nc.dram_tensor("WpT1", [C, C], BF16, kind="ExternalInput")
    bpT = nc.dram_tensor("bpT", [C, 1], F32, kind="ExternalInput")
    outT = nc.dram_tensor("outT", [C, NQ], F32, kind="ExternalOutput")

    with tile.TileContext(nc) as tc:
        with (
            tc.tile_pool(name="const", bufs=1) as cpool,
            tc.tile_pool(name="data", bufs=1) as dpool,
        ):
            w_q = cpool.tile([C, C], BF16, name="w_q")
            w_k = cpool.tile([C, C], BF16, name="w_k")
            w_v = cpool.tile([C, C], BF16, name="w_v")
            ident = cpool.tile([C, C], FP8E5, name="ident")
            sel = cpool.tile([C, C], BF16, name="sel")
            w_p0 = cpool.tile([C, C], BF16, name="w_p0")
            w_p1 = cpool.tile([C, C], BF16, name="w_p1")
            bp_sb = cpool.tile([C, 1], F32, name="bp_sb")
            b0_sb = cpool.tile([C, 1], F32, name="b0_sb")
            nc.sync.dma_start(w_q[:], WqT[:])
            nc.sync.dma_start(w_k[:], WkT[:])
            nc.sync.dma_start(w_v[:], WvT[:])
            nc.sync.dma_start(ident[:], IDENT[:])
            nc.sync.dma_start(sel[:], SEL[:])
            nc.sync.dma_start(w_p0[:], WpT0[:])
            nc.sync.dma_start(w_p1[:], WpT1[:])
            nc.sync.dma_start(bp_sb[:], bpT[:])
            nc.gpsimd.memset(b0_sb[:], B0)

            xT_sb = dpool.tile([C, N], BF16, name="xT_sb")
            xqT_sb = dpool.tile([C, NQ], F32, name="xqT_sb")
            nc.sync.dma_start(xT_sb[:], xT[:])
            nc.sync.dma_start(xqT_sb[:], xqT[:])
            at_sb = []
            for kb in range(KB):
                t = dpool.tile([128, 2 * NQ], FP8E5, name=f"at{kb}")
                nc.sync.dma_start(t[:], AT16[kb * 128:(kb + 1) * 128, :])
                at_sb.append(t)

            kT_sb = dpool.tile([C, N], BF16, name="kT_sb")
            qT_sb = dpool.tile([C, NQ], BF16, name="qT_sb")
            vaug_sb = dpool.tile([128, KB * H * 64], BF16, name="vaug_sb")
            nc.gpsimd.memset(vaug_sb[:], 1.0)

            with tc.tile_pool(name="pjps", bufs=2, space="PSUM") as pjps:
                for ch in range(N // 512):
                    ps = pjps.tile([C, 512], F32, name=f"pk{ch}", tag="pj")
                    nc.tensor.matmul(
                        ps[:], w_k[:], xT_sb[:, ch * 512:(ch + 1) * 512]
                    )
                    nc.scalar.activation(
                        kT_sb[:, ch * 512:(ch + 1) * 512], ps[:], CPY
                    )
                for ch in range(NQ // 512):
                    ps = pjps.tile([C, 512], F32, name=f"pq{ch}", tag="pj")
                    nc.tensor.matmul(
                        ps[:], w_q[:], xT_sb[:, ch * 512:(ch + 1) * 512]
                    )
                    nc.scalar.activation(
                        qT_sb[:, ch * 512:(ch + 1) * 512], ps[:], CPY
                    )
                for kb in range(KB):
                    ps = pjps.tile([128, C], F32, name=f"pv{kb}", tag="pj")
                    nc.tensor.matmul(
                        ps[:], xT_sb[:, kb * 128:(kb + 1) * 128], w_v[:]
                    )
                    dst = vaug_sb[:, kb * 256:(kb + 1) * 256].rearrange(
                        "p (h x) -> p h x", x=64
                    )[:, :, 0:32]
                    src = ps[:].rearrange("p (h d) -> p h d", d=32)
                    nc.vector.tensor_copy(dst, src)

            with (
                tc.tile_pool(name="sps", bufs=3, space="PSUM") as sps,
                tc.tile_pool(name="accps", bufs=1, space="PSUM") as accps,
                tc.tile_pool(name="ppool", bufs=6) as ppool,
                tc.tile_pool(name="epool", bufs=2) as epool,
            ):
                deferred_out = [None]

                def emit_out_stage(qc, acc_ps):
                    qs = slice(qc * 512, (qc + 1) * 512)
                    rr_sb = epool.tile([128, 1024], BF16, name=f"rr{qc}", tag="rr")
                    with nc.allow_low_precision(
                        reason="1/den broadcast; bf16 ample for softmax"
                    ):
                        nc.vector.reciprocal(rr_sb[:], acc_ps[:])
                    bc_sb = epool.tile([128, 1024], F32, name=f"bc{qc}", tag="bc")
                    for bh in range(2):
                        bc_ps = sps.tile(
                            [128, 512], F32, name=f"bcp{qc}_{bh}", tag="s"
                        )
                        nc.tensor.matmul(
                            bc_ps[:], sel[:], rr_sb[:, bh * 512:(bh + 1) * 512]
                        )
                        nc.scalar.activation(
                            bc_sb[:, bh * 512:(bh + 1) * 512], bc_ps[:], CPY
                        )
                    asc_sb = epool.tile(
                        [128, 1024], BF16, name=f"asc{qc}", tag="asc"
                    )
                    nc.vector.scalar_tensor_tensor(
                        asc_sb[:], acc_ps[:], 0.0, bc_sb[:], ADD, MULT
                    )
                    o2 = sps.tile([128, 512], F32, name=f"o2_{qc}", tag="s")
                    for bh, w_pb in enumerate((w_p0, w_p1)):
                        nc.tensor.matmul(
                            o2[:],
                            w_pb[:],
                            asc_sb[:, bh * 512:(bh + 1) * 512],
                            start=(bh == 0),
                            stop=(bh == 1),
                        )
                    o_sb = epool.tile([128, 512], F32, name=f"ot{qc}", tag="ot")
                    nc.vector.scalar_tensor_tensor(
                        o_sb[:], o2[:], bp_sb[:], xqT_sb[:, qs], ADD, ADD
                    )
                    nc.sync.dma_start(outT[:, qs], o_sb[:])

                for qc in range(QC):
                    qs = slice(qc * 512, (qc + 1) * 512)
                    aq = slice(qc * 1024, (qc + 1) * 1024)
                    acc_ps = accps.tile([128, 1024], F32, name=f"acc{qc}", tag="acc")
                    last_score_mm = [None]

                    def emit_pv_one(
                        kb, hp, p_ap, hh, acc_ps=acc_ps, lsm=last_score_mm
                    ):
                        h = hp * 2 + hh
                        m, b = h % 2, h // 2
                        mm = nc.tensor.matmul(
                            acc_ps[64 * m:64 * (m + 1), b * 512:(b + 1) * 512],
                            vaug_sb[:, kb * 256 + h * 64:kb * 256 + (h + 1) * 64],
                            p_ap(hh),
                            start=(kb == 0),
                            stop=(kb == KB - 1),
                            tile_position=(0, 64 * m),
                            skip_group_check=True,
                        )
                        if lsm[0] is not None:
                            add_dep_helper(
                                mm.ins, lsm[0], sync=False,
                                reason="sw-pipeline PE order",
                            )

                    pending = []

                    def drain_pair():
                        (k0, h0, p0), (k1, h1, p1) = pending[0], pending[1]
                        del pending[0:2]
                        for (kx, hx, px, hh) in (
                            (k0, h0, p0, 0), (k1, h1, p1, 1),
                            (k0, h0, p0, 1), (k1, h1, p1, 0),
                        ):
                            emit_pv_one(kx, hx, px, hh)

                    def drain_pv():
                        while len(pending) >= 4:
                            drain_pair()

                    for kb in range(KB):
                        ks = slice(kb * 128, (kb + 1) * 128)
                        cls = [_tile_class(qc, kb, hp) for hp in range(2)]
                        s_tiles = [
                            sps.tile(
                                [128, 1024], F32, name=f"s{qc}_{kb}_{hp}", tag="s"
                            )
                            for hp in range(2)
                        ]
                        for h in range(H):
                            hs = slice(32 * h, 32 * (h + 1))
                            hp, hh = h // 2, h % 2
                            mm = nc.tensor.matmul(
                                s_tiles[hp][:, hh * 512:(hh + 1) * 512],
                                kT_sb[hs, ks],
                                qT_sb[hs, qs],
                                tile_position=(32 * h, 0),
                                start=True,
                                stop=(cls[hp] != "C"),
                            )
                            last_score_mm[0] = mm.ins
                        for hp in range(2):
                            if cls[hp] != "C":
                                continue
                            for bh in range(2):
                                mm = nc.tensor.matmul(
                                    s_tiles[hp][:, bh * 512:(bh + 1) * 512],
                                    ident[:],
                                    at_sb[kb][
                                        :,
                                        qc * 1024 + bh * 512:
                                        qc * 1024 + (bh + 1) * 512,
                                    ],
                                    start=False,
                                    stop=True,
                                )
                                last_score_mm[0] = mm.ins
                        drain_pv()
                        for hp in range(2):
                            s_ps = s_tiles[hp]
                            if cls[hp] == "C":
                                p_sb = ppool.tile(
                                    [128, 1024], BF16,
                                    name=f"p{qc}_{kb}_{hp}", tag="p",
                                )
                                nc.scalar.activation(
                                    p_sb[:], s_ps[:], EXP,
                                    scale=float(1.0 / ALPHA), bias=b0_sb[:],
                                )
                                p_ap = (
                                    lambda hh, p_sb=p_sb:
                                    p_sb[:, hh * 512:(hh + 1) * 512]
                                )
                            else:
                                p_sb = ppool.tile(
                                    [128, 1024], I16,
                                    name=f"p{qc}_{kb}_{hp}", tag="p",
                                )
                                nc.vector.scalar_tensor_tensor(
                                    p_sb[:], s_ps[:], MAGIC,
                                    at_sb[kb][:, aq], ADD, ADD,
                                )
                                p_ap = (
                                    lambda hh, p_sb=p_sb:
                                    p_sb[:, hh * 512:(hh + 1) * 512]
                                    .bitcast(mybir.dt.bfloat16)
                                )
                            pending.append((kb, hp, p_ap))
                        if kb == 1 and deferred_out[0]:
                            emit_out_stage(*deferred_out[0])
                            deferred_out[0] = None
                    while pending:
                        drain_pair()
                    deferred_out[0] = (qc, acc_ps)

                if deferred_out[0]:
                    emit_out_stage(*deferred_out[0])
                    deferred_out[0] = None

    nc.compile()
    return nc


def _prep_in_maps(x, A, Wq, Wkv, Wp, bp):
    import ml_dtypes

    bf16 = ml_dtypes.bfloat16
    fp8 = ml_dtypes.float8_e5m2
    x = np.asarray(x, np.float32)
    A = np.asarray(A)
    Wq = np.asarray(Wq, np.float32)
    Wkv = np.asarray(Wkv, np.float32)
    Wp = np.asarray(Wp, np.float32)
    bp = np.asarray(bp, np.float32)

    wq = np.ascontiguousarray((Wq * SCALE * ALPHA).T).astype(bf16)
    wk = np.ascontiguousarray(Wkv[:C].T).astype(bf16)
    wv = np.ascontiguousarray(Wkv[C:].T).astype(bf16)
    bpT = np.ascontiguousarray(bp.reshape(C, 1))
    ident = np.eye(C, dtype=np.float32).astype(fp8)

    # selector matmul constant: bcast[j, q] = rr[64*(j//64)+32, q]
    sel = np.zeros((C, C), np.float32)
    for j in range(C):
        sel[64 * (j // 64) + 32, j] = 1.0
    sel = sel.astype(bf16)
    # Wp.T rows rearranged to the PV accumulator layout (denominator rows = 0)
    wpT = Wp.T
    wpb = []
    for b in range(2):
        w = np.zeros((C, C), np.float32)
        for r in range(C):
            d = r % 64
            if d < 32:
                w[r, :] = wpT[32 * (2 * b + r // 64) + d, :]
        wpb.append(np.ascontiguousarray(w).astype(bf16))

    a16_full = (A.astype(np.float32) - 1.0) * AMASK

    in_maps = []
    for core in range(8):
        b, s = divmod(core, 2)
        sl = slice(s * NQ, (s + 1) * NQ)
        xTb = np.ascontiguousarray(x[b].T)
        a16 = a16_full[sl, :].T  # [N, NQ]
        at16 = np.concatenate(
            [a16[:, 0:512], a16[:, 0:512], a16[:, 512:1024], a16[:, 512:1024]],
            axis=1,
        ).astype(fp8)
        in_maps.append(
            {
                "xT": xTb.astype(bf16),
                "xqT": np.ascontiguousarray(xTb[:, sl]),
                "AT16": at16,
                "WqT": wq,
                "WkT": wk,
                "WvT": wv,
                "IDENT": ident,
                "SEL": sel,
                "WpT0": wpb[0],
                "WpT1": wpb[1],
                "bpT": bpT,
            }
        )
    return in_maps


def kernel(x, A, Wq, Wkv, Wp, bp):
    from concourse.bass_utils import run_bass_kernel_spmd

    if "nc" not in _CACHE:
        _CACHE["nc"] = _build()
    nc = _CACHE["nc"]
    in_maps = _prep_in_maps(x, A, Wq, Wkv, Wp, bp)
    res = run_bass_kernel_spmd(nc, in_maps, list(range(8)))
    out = np.empty((B, N, C), np.float32)
    for core in range(8):
        b, s = divmod(core, 2)
        out[b, s * NQ:(s + 1) * NQ, :] = res.results[core]["outT"].T
    return out
